# revision 27
# baseline (speedup 1.0000x reference)
"""MoE (MiniMax decoder MLP) Trainium2 kernel — expert-parallel across 8 NeuronCores.

Strategy (per the expert-parallel sharding hint):
  - Host computes the router (softmax + top-2 + renormalize) — this IS the
    sharding decision — and dispatches each token's activation row to the
    core(s) owning its selected expert(s).
  - Core e holds expert e's weights [H,I]/[H,I]/[I,H] and computes
    silu(x @ Wg) * (x @ Wu) @ Wd for its routed tokens, scaling rows by the
    renormalized combine weight on-device.
  - Capacity is fixed at the balanced ideal C = T*K/8 = 2048 tokens/core so
    every core runs the identical minimal-FLOP program (4 uniform 512-token
    windows, 16 full m-tiles). The few token-expert pairs beyond a hot
    expert's capacity (~0.6% of pairs for this router distribution) are
    computed on host in exact fp32 during the combine step.
  - Host scatter-adds the per-expert outputs back into the full [T, H] output.

Compute is done in fp16 on the TensorEngine (fp32 PSUM accumulation); output
is returned as fp16 and upcast host-side (error ~5e-4 « the fp32 tolerance).

Schedule notes (from perfetto traces of this kernel; HW-measured):
  - Engines are blocked ~6-10us at kernel start (start barrier + per-engine
    instruction-load preamble) — no DMA can begin before that. The PE
    warm-up loop (garbage matmuls on a memset tile) keeps the PE busy from
    ~7us until the first real matmul's data lands so the HAM clock gate
    reaches 8/8 early.
  - Startup is HBM-bound (~358GB/s combined over all rings; ring count adds
    nothing) so GLOBAL arrival order must match consumption order: the sync
    HWDGE ring carries [gate/up i-block pairs, wd, xt w1] in consumption
    order (ring FIFO = priority); the scalar ring carries only window-0's
    activations + cw early. The first 4 weight pairs are split into 0.25MB
    halves so any early stall stays under the 3.4us HAM re-throttle window.
  - matmul1 is i-outer/k-inner: 16 consecutive matmuls accumulate into the
    SAME PSUM bank — cycling banks per-MM costs +43ns/MM (+20%), measured.
  - Down-proj outputs are staged per m-tile into a [128, H] fp16 slab and
    written with ONE dma_start (128 descriptors of 4KB — descriptor count
    is what bounds HWDGE trigger time, ~6ns each), issued on the sync ring
    so triggers never delay silu on the scalar engine. The last window
    instead issues per-h-window DMAs so the final transfer trailing the
    last matmul is small (~128KB + ~2us HBM write receipt).
"""

import os
import sys

import numpy as np

_EXTRA_PATHS = [
    "/root/.axon_site",
    "/root/.axon_site/_ro/trn_rl_repo",
    "/root/.axon_site/_ro/pypackages",
    "/opt/trn_rl_repo",
    "/opt/pypackages",
]
try:
    import concourse.bass  # noqa: F401
except ImportError:  # pragma: no cover
    sys.path[:0] = [p for p in _EXTRA_PATHS if p not in sys.path]

B, S, H = 4, 2048, 2048
I = 1408  # expert intermediate size
E = 8  # num experts
K = 2  # experts per token
N_CORES = 8

KT = H // 128  # 16 contraction tiles over H
IT = I // 128  # 11 tiles over I
HW = H // 512  # 4 output windows over H
WBLK = KT * 128  # free-dim span of one i-block in the wg/wu SBUF image
C = 2048  # per-core token capacity (the balanced ideal T*K/N_CORES)
MT = C // 128  # 16 token m-tiles
NWIN = C // 512  # 4 token windows
N_WARM = 15  # warm-up matmuls (~9.5us at the cold 1.2GHz clock)

_NC_CACHE = {}


def _build_nc():
    """Build + compile the per-core expert MLP program (capacity C tokens)."""
    import concourse.mybir as mybir
    import concourse.tile as tile
    from concourse import bacc

    fp32 = mybir.dt.float32
    fp16 = mybir.dt.float16
    mult = mybir.AluOpType.mult
    silu_fn = mybir.ActivationFunctionType.Silu

    nc = bacc.Bacc("TRN2", target_bir_lowering=False, debug=False, num_devices=N_CORES)

    # All inputs pre-swizzled to SBUF-image layouts (see kernel() below).
    xt = nc.dram_tensor("xt", [128, KT * C], fp16, kind="ExternalInput")
    wg = nc.dram_tensor("wg", [128, IT * WBLK], fp16, kind="ExternalInput")
    wu = nc.dram_tensor("wu", [128, IT * WBLK], fp16, kind="ExternalInput")
    wd = nc.dram_tensor("wd", [128, IT * H], fp16, kind="ExternalInput")
    cw = nc.dram_tensor("cw", [128, MT], fp32, kind="ExternalInput")
    out = nc.dram_tensor("out", [C, H], fp16, kind="ExternalOutput")

    with tile.TileContext(nc) as tc:
        with (
            tc.tile_pool(name="wpool", bufs=1) as wpool,
            tc.tile_pool(name="xpool", bufs=2) as xpool,
            tc.tile_pool(name="gpool", bufs=2) as gpool,
            tc.tile_pool(name="spool", bufs=2) as spool,
            tc.tile_pool(name="opool", bufs=3) as opool,
            tc.tile_pool(name="cwpool", bufs=1) as cwpool,
            tc.tile_pool(name="warm", bufs=1) as warm,
            tc.tile_pool(name="pgp", bufs=2, space="PSUM") as pgp,
            tc.tile_pool(name="pup", bufs=2, space="PSUM") as pup,
            tc.tile_pool(name="pop", bufs=4, space="PSUM") as pop,
        ):
            # PE warm-up on a memset tile: keeps the PE continuously busy
            # from ~0.5us until the first real matmul's data has landed, so
            # the HAM clock gate is at 8/8 (2.4GHz) for all real work.
            # Output goes to a pop-pool PSUM buf (reused by mm2 much later).
            wsrc = warm.tile([128, 512], fp16, name="wsrc", tag="wsrc")
            nc.vector.memset(wsrc[:], 1.0)
            pw = pop.tile([128, 512], fp32, name="pw", tag="po")
            for r in range(N_WARM):
                nc.tensor.matmul(
                    pw[:],
                    wsrc[:, :128],
                    wsrc[:],
                    start=(r == 0),
                    stop=(r == N_WARM - 1),
                )

            # The weight streams are split across BOTH HWDGE rings so they
            # transfer in parallel (a single ring sustains only ~150-330GB/s
            # with 4KB-line descriptors; mm1 consumes gate+up at 145GB/s):
            # gate blocks on the sync ring, up blocks on the scalar ring
            # interleaved behind window-0's activation chunks. Ring FIFO
            # order = transfer priority, matching mm1's consumption order.
            def dma_xt_window(o, nsplit):
                t = xpool.tile([128, KT * 512], fp16, name="xt_sb", tag="xt_sb")
                span = KT * 512
                step = span // nsplit
                chunks = []
                for s0 in range(0, span, step):
                    chunks.append(
                        (t[:, s0 : s0 + step], xt.ap()[:, KT * o + s0 : KT * o + s0 + step])
                    )
                return t, chunks

            wg_sb = wpool.tile([128, IT * WBLK], fp16, name="wg_sb", tag="wg_sb")
            wu_sb = wpool.tile([128, IT * WBLK], fp16, name="wu_sb", tag="wu_sb")
            wd_sb = wpool.tile([128, IT * H], fp16, name="wd_sb", tag="wd_sb")
            cw_sb = cwpool.tile([128, MT], fp32, name="cw_sb", tag="cw_sb")

            xt0_sb, xt0_chunks = dma_xt_window(0, nsplit=4)
            # Startup is HBM-bandwidth-bound (~358GB/s combined across
            # rings), so GLOBAL arrival order must match consumption order:
            # the sync ring carries [gate/up pairs, wd, xt w1] back-to-back
            # — mm1 consumes a 1MB pair every 3.46us, wd is needed ~90us
            # in, xt w1 ~85us in. The scalar ring carries only window-0's
            # activations + cw early (~2MB, done by ~13us) so it stops
            # competing once the weight stream is the critical path.
            # (xt w2/w3 stay on scalar: their triggers block on xpool buf
            # release until ~90/160us, which would head-of-line block the
            # sync ring's out slabs.)
            for c in xt0_chunks:
                nc.scalar.dma_start(*c)
            # First 4 pairs in 0.25MB halves: early stalls then quantize at
            # <2us — a single stall >3.4us re-throttles the HAM clock gate
            # to 1.2GHz and costs another 3.4us of half-rate to re-warm.
            for i in range(IT):
                nhalf = 2 if i < 4 else 1
                hstep = WBLK // nhalf
                for w_sb, w in ((wg_sb, wg), (wu_sb, wu)):
                    for s0 in range(i * WBLK, (i + 1) * WBLK, hstep):
                        nc.sync.dma_start(
                            w_sb[:, s0 : s0 + hstep], w.ap()[:, s0 : s0 + hstep]
                        )
            nc.scalar.dma_start(cw_sb[:], cw.ap()[:])
            for a, b in ((0, 6), (6, IT)):
                nc.sync.dma_start(
                    wd_sb[:, a * H : b * H], wd.ap()[:, a * H : b * H]
                )

            def emit_matmul1(xt_sb):
                """silu(x@Wg) * (x@Wu) for one 512-token window -> gated^T.

                i-outer, k-inner: 16 consecutive matmuls accumulate into the
                SAME PSUM bank — cycling banks per-MM costs +43ns/MM (20%)
                in the PE pipeline, measured.
                """
                gated = []
                for i in range(IT):
                    pg = pgp.tile([128, 512], fp32, name="pg", tag="pg")
                    pu = pup.tile([128, 512], fp32, name="pu", tag="pu")
                    for k in range(KT):
                        nc.tensor.matmul(
                            pg[:],
                            wg_sb[:, i * WBLK + k * 128 : i * WBLK + (k + 1) * 128],
                            xt_sb[:, k * 512 : (k + 1) * 512],
                            start=(k == 0),
                            stop=(k == KT - 1),
                        )
                    for k in range(KT):
                        nc.tensor.matmul(
                            pu[:],
                            wu_sb[:, i * WBLK + k * 128 : i * WBLK + (k + 1) * 128],
                            xt_sb[:, k * 512 : (k + 1) * 512],
                            start=(k == 0),
                            stop=(k == KT - 1),
                        )
                    act = spool.tile([128, 512], fp32, name="act", tag="act")
                    nc.scalar.activation(act[:], pg[:], silu_fn)
                    g = gpool.tile([128, 512], fp16, name=f"g{i}", tag=f"g{i}")
                    nc.vector.tensor_tensor(g[:], act[:], pu[:], mult)
                    gated.append(g)
                return gated

            def emit_matmul2(wi, gated, last=False):
                # Down-proj: out[tokens, H] accumulated over I, then scaled by
                # the per-token combine weight into an fp16 staging slab,
                # written back with one DMA per m-tile (sync ring). The last
                # window issues per-h DMAs instead so the final transfer
                # trailing the last matmul is small.
                for m in range(4):
                    mg = wi * 4 + m
                    ob = opool.tile([128, HW * 512], fp16, name="ob", tag="ob")
                    for h in range(HW):
                        po = pop.tile([128, 512], fp32, name="po", tag="po")
                        for i in range(IT):
                            nc.tensor.matmul(
                                po[:],
                                gated[i][:, m * 128 : (m + 1) * 128],
                                wd_sb[:, i * H + h * 512 : i * H + (h + 1) * 512],
                                start=(i == 0),
                                stop=(i == IT - 1),
                            )
                        nc.vector.tensor_scalar_mul(
                            ob[:, h * 512 : (h + 1) * 512], po[:], cw_sb[:, mg : mg + 1]
                        )
                        if last:
                            eng = nc.scalar if h % 2 else nc.sync
                            eng.dma_start(
                                out.ap()[
                                    mg * 128 : (mg + 1) * 128, h * 512 : (h + 1) * 512
                                ],
                                ob[:, h * 512 : (h + 1) * 512],
                            )
                    if not last:
                        nc.sync.dma_start(
                            out.ap()[mg * 128 : (mg + 1) * 128, :], ob[:]
                        )

            # Window pipeline: matmul2 of window t is emitted after matmul1 of
            # window t+1 (gpool bufs=2 keeps both windows' gated tiles live),
            # so the start-up down-matmuls don't stall on the wd load.
            pending = None
            for wi in range(NWIN):
                if wi == 0:
                    xt_sb = xt0_sb
                else:
                    xt_sb, xchunks = dma_xt_window(wi * 512, nsplit=1)
                    for c in xchunks:
                        # w1 rides the sync ring right behind wd (its xpool
                        # buf is free at t0); w2/w3 triggers block on buf
                        # release so they go on the scalar ring.
                        (nc.sync if wi == 1 else nc.scalar).dma_start(*c)
                gated = emit_matmul1(xt_sb)
                if pending is not None:
                    emit_matmul2(*pending)
                pending = (wi, gated)
            emit_matmul2(*pending, last=True)

    nc.compile()
    return nc


def kernel(
    hidden_states: np.ndarray,
    gate_w: np.ndarray,
    w_gate: np.ndarray,
    w_up: np.ndarray,
    w_down: np.ndarray,
) -> np.ndarray:
    from concourse.bass_utils import run_bass_kernel_spmd

    x = np.asarray(hidden_states, dtype=np.float32).reshape(-1, H)
    gate_w = np.asarray(gate_w, dtype=np.float32)
    w_gate = np.asarray(w_gate, dtype=np.float32)
    w_up = np.asarray(w_up, dtype=np.float32)
    w_down = np.asarray(w_down, dtype=np.float32)
    T = x.shape[0]

    # Router (the sharding decision): softmax over experts, top-2, renormalize.
    logits = x @ gate_w.T
    logits -= logits.max(axis=-1, keepdims=True)
    ex = np.exp(logits)
    probs = ex / ex.sum(axis=-1, keepdims=True)
    topk_i = np.argpartition(-probs, K - 1, axis=-1)[:, :K]  # [T, K]
    topk_w = np.take_along_axis(probs, topk_i, axis=-1)
    denom = topk_w.sum(axis=-1)  # [T]

    sels, cws, overflow = [], [], []
    for e in range(E):
        sel = np.nonzero((topk_i == e).any(axis=1))[0]
        cw_e = probs[sel, e] / denom[sel]
        if len(sel) > C:
            overflow.append((e, sel[C:], cw_e[C:]))
            sel, cw_e = sel[:C], cw_e[:C]
        sels.append(sel)
        cws.append(cw_e)

    if "nc" not in _NC_CACHE:
        _NC_CACHE["nc"] = _build_nc()
    nc = _NC_CACHE["nc"]

    # Dispatch: gather each expert's tokens (transposed, fp16) + weights,
    # swizzled into the SBUF-image layouts the kernel's DMAs expect.
    xt_full = np.ascontiguousarray(x.T.astype(np.float16))  # [H, T]

    def swz_w(w):  # [H, I] -> [128, IT*KT*128] i-block-major image
        return np.ascontiguousarray(
            w.astype(np.float16)
            .reshape(KT, 128, IT, 128)
            .transpose(1, 2, 0, 3)
            .reshape(128, IT * KT * 128)
        )

    def swz_wd(w):  # [I, H] -> [128, IT*H] i-block-major image
        return np.ascontiguousarray(
            w.astype(np.float16).reshape(IT, 128, H).transpose(1, 0, 2).reshape(128, IT * H)
        )

    def swz_xt(xpad):  # [H, C] -> [128, KT*C] window-major image
        blocks = [
            xpad[:, o : o + 512].reshape(KT, 128, 512).transpose(1, 0, 2).reshape(128, -1)
            for o in range(0, C, 512)
        ]
        return np.ascontiguousarray(np.concatenate(blocks, axis=1))

    in_maps = []
    for e in range(E):
        sel = sels[e]
        xpad = np.zeros((H, C), dtype=np.float16)
        xpad[:, : len(sel)] = xt_full[:, sel]
        cw_e = np.zeros((128, MT), dtype=np.float32)
        cw_flat = np.zeros(MT * 128, dtype=np.float32)
        cw_flat[: len(sel)] = cws[e]
        cw_e[:] = cw_flat.reshape(MT, 128).T
        in_maps.append(
            {
                "xt": swz_xt(xpad),
                "wg": swz_w(w_gate[e]),
                "wu": swz_w(w_up[e]),
                "wd": swz_wd(w_down[e]),
                "cw": cw_e,
            }
        )

    trace = bool(os.environ.get("BASS_MOE_TRACE"))
    res = run_bass_kernel_spmd(
        nc, in_maps, core_ids=list(range(N_CORES)), trace=trace
    )
    if trace and res.exec_time_ns is not None:
        print(f"HW exec time: {res.exec_time_ns} ns")

    # Combine: scatter-add each expert's (already weight-scaled) rows.
    out_full = np.zeros((T, H), dtype=np.float32)
    for e in range(E):
        sel = sels[e]
        out_full[sel] += res.results[e]["out"][: len(sel)].astype(np.float32)
    # Token-expert pairs beyond a hot expert's capacity: exact fp32 on host
    # (~0.6% of pairs for this router distribution).
    for e, sel, cw_e in overflow:
        xs = x[sel]
        g = xs @ w_gate[e]
        gated = (g / (1.0 + np.exp(-g))) * (xs @ w_up[e])
        out_full[sel] += cw_e[:, None] * (gated @ w_down[e])
    return out_full.reshape(B, S, H)


# revision 29
# speedup vs baseline: 1.0419x; 1.0419x over previous
"""MoE (MiniMax decoder MLP) Trainium2 kernel — expert-parallel across 8 NeuronCores.

Strategy (per the expert-parallel sharding hint):
  - Host computes the router (softmax + top-2 + renormalize) — this IS the
    sharding decision — and dispatches each token's activation row to the
    core(s) owning its selected expert(s).
  - Core e holds expert e's weights [H,I]/[H,I]/[I,H] and computes
    silu(x @ Wg) * (x @ Wu) @ Wd for its routed tokens, scaling rows by the
    renormalized combine weight on-device.
  - Capacity is fixed at the balanced ideal C = T*K/8 = 2048 tokens/core so
    every core runs the identical minimal-FLOP program (4 uniform 512-token
    windows, 16 full m-tiles). The few token-expert pairs beyond a hot
    expert's capacity (~0.6% of pairs for this router distribution) are
    computed on host in exact fp32 during the combine step.
  - Host scatter-adds the per-expert outputs back into the full [T, H] output.

Compute is done in fp16 on the TensorEngine (fp32 PSUM accumulation); output
is returned as fp16 and upcast host-side (error ~5e-4 « the fp32 tolerance).

Schedule notes (from perfetto traces of this kernel; HW-measured):
  - Engines are blocked ~6-10us at kernel start (start barrier + per-engine
    instruction-load preamble) — no DMA can begin before that. The PE
    warm-up loop (garbage matmuls on a memset tile) keeps the PE busy from
    ~7us until the first real matmul's data lands so the HAM clock gate
    reaches 8/8 early.
  - Startup is HBM-bound (~358GB/s combined over all rings; ring count adds
    nothing) so GLOBAL arrival order must match consumption order: the sync
    HWDGE ring carries [gate/up i-block pairs, wd, xt w1] in consumption
    order (ring FIFO = priority); the scalar ring carries only window-0's
    activations + cw early. The first 4 weight pairs are split into 0.25MB
    halves so any early stall stays under the 3.4us HAM re-throttle window.
  - matmul1 is i-outer/k-inner: 16 consecutive matmuls accumulate into the
    SAME PSUM bank — cycling banks per-MM costs +43ns/MM (+20%), measured.
  - Down-proj outputs are staged per m-tile into a [128, H] fp16 slab and
    written with ONE dma_start (128 descriptors of 4KB — descriptor count
    is what bounds HWDGE trigger time, ~6ns each), issued on the sync ring
    so triggers never delay silu on the scalar engine. The last window
    instead issues per-h-window DMAs so the final transfer trailing the
    last matmul is small (~128KB + ~2us HBM write receipt).
"""

import os
import sys

import numpy as np

_EXTRA_PATHS = [
    "/root/.axon_site",
    "/root/.axon_site/_ro/trn_rl_repo",
    "/root/.axon_site/_ro/pypackages",
    "/opt/trn_rl_repo",
    "/opt/pypackages",
]
try:
    import concourse.bass  # noqa: F401
except ImportError:  # pragma: no cover
    sys.path[:0] = [p for p in _EXTRA_PATHS if p not in sys.path]

B, S, H = 4, 2048, 2048
I = 1408  # expert intermediate size
E = 8  # num experts
K = 2  # experts per token
N_CORES = 8

KT = H // 128  # 16 contraction tiles over H
IT = I // 128  # 11 tiles over I
HW = H // 512  # 4 output windows over H
WBLK = KT * 128  # free-dim span of one i-block in the wg/wu SBUF image
C = 2048  # per-core token capacity (the balanced ideal T*K/N_CORES)
MT = C // 128  # 16 token m-tiles
NWIN = C // 512  # 4 token windows
N_WARM = 15  # warm-up matmuls (~9.5us at the cold 1.2GHz clock)

_NC_CACHE = {}


def _build_nc():
    """Build + compile the per-core expert MLP program (capacity C tokens)."""
    import concourse.mybir as mybir
    import concourse.tile as tile
    from concourse import bacc

    fp32 = mybir.dt.float32
    fp16 = mybir.dt.float16
    mult = mybir.AluOpType.mult
    silu_fn = mybir.ActivationFunctionType.Silu

    nc = bacc.Bacc("TRN2", target_bir_lowering=False, debug=False, num_devices=N_CORES)

    # All inputs pre-swizzled to SBUF-image layouts (see kernel() below).
    xt = nc.dram_tensor("xt", [128, KT * C], fp16, kind="ExternalInput")
    wg = nc.dram_tensor("wg", [128, IT * WBLK], fp16, kind="ExternalInput")
    wu = nc.dram_tensor("wu", [128, IT * WBLK], fp16, kind="ExternalInput")
    wd = nc.dram_tensor("wd", [128, IT * H], fp16, kind="ExternalInput")
    cw = nc.dram_tensor("cw", [128, MT], fp32, kind="ExternalInput")
    out = nc.dram_tensor("out", [C, H], fp16, kind="ExternalOutput")

    with tile.TileContext(nc) as tc:
        with (
            tc.tile_pool(name="wpool", bufs=1) as wpool,
            tc.tile_pool(name="xpool", bufs=2) as xpool,
            tc.tile_pool(name="gpool", bufs=2) as gpool,
            tc.tile_pool(name="spool", bufs=2) as spool,
            tc.tile_pool(name="opool", bufs=3) as opool,
            tc.tile_pool(name="cwpool", bufs=1) as cwpool,
            tc.tile_pool(name="warm", bufs=1) as warm,
            tc.tile_pool(name="pgp", bufs=2, space="PSUM") as pgp,
            tc.tile_pool(name="pup", bufs=2, space="PSUM") as pup,
            tc.tile_pool(name="pop", bufs=4, space="PSUM") as pop,
        ):
            # PE warm-up on a memset tile: keeps the PE continuously busy
            # from ~0.5us until the first real matmul's data has landed, so
            # the HAM clock gate is at 8/8 (2.4GHz) for all real work.
            # Output goes to a pop-pool PSUM buf (reused by mm2 much later).
            wsrc = warm.tile([128, 512], fp16, name="wsrc", tag="wsrc")
            nc.vector.memset(wsrc[:], 1.0)
            pw = pop.tile([128, 512], fp32, name="pw", tag="po")
            for r in range(N_WARM):
                nc.tensor.matmul(
                    pw[:],
                    wsrc[:, :128],
                    wsrc[:],
                    start=(r == 0),
                    stop=(r == N_WARM - 1),
                )

            # The weight streams are split across BOTH HWDGE rings so they
            # transfer in parallel (a single ring sustains only ~150-330GB/s
            # with 4KB-line descriptors; mm1 consumes gate+up at 145GB/s):
            # gate blocks on the sync ring, up blocks on the scalar ring
            # interleaved behind window-0's activation chunks. Ring FIFO
            # order = transfer priority, matching mm1's consumption order.
            def dma_xt_window(o, nsplit):
                t = xpool.tile([128, KT * 512], fp16, name="xt_sb", tag="xt_sb")
                span = KT * 512
                step = span // nsplit
                chunks = []
                for s0 in range(0, span, step):
                    chunks.append(
                        (t[:, s0 : s0 + step], xt.ap()[:, KT * o + s0 : KT * o + s0 + step])
                    )
                return t, chunks

            wg_sb = wpool.tile([128, IT * WBLK], fp16, name="wg_sb", tag="wg_sb")
            wu_sb = wpool.tile([128, IT * WBLK], fp16, name="wu_sb", tag="wu_sb")
            wd_sb = wpool.tile([128, IT * H], fp16, name="wd_sb", tag="wd_sb")
            cw_sb = cwpool.tile([128, MT], fp32, name="cw_sb", tag="cw_sb")

            xt0_sb, xt0_chunks = dma_xt_window(0, nsplit=4)
            # Startup is HBM-bandwidth-bound (~358GB/s combined across
            # rings), so GLOBAL arrival order must match consumption order:
            # the sync ring carries [gate/up pairs, wd, xt w1] back-to-back
            # — mm1 consumes a 1MB pair every 3.46us, wd is needed ~90us
            # in, xt w1 ~85us in. The scalar ring carries only window-0's
            # activations + cw early (~2MB, done by ~13us) so it stops
            # competing once the weight stream is the critical path.
            # (xt w2/w3 stay on scalar: their triggers block on xpool buf
            # release until ~90/160us, which would head-of-line block the
            # sync ring's out slabs.)
            for c in xt0_chunks:
                nc.scalar.dma_start(*c)
            # First 4 pairs in 0.25MB halves: early stalls then quantize at
            # <2us — a single stall >3.4us re-throttles the HAM clock gate
            # to 1.2GHz and costs another 3.4us of half-rate to re-warm.
            for i in range(IT):
                nhalf = 2 if i < 4 else 1
                hstep = WBLK // nhalf
                for w_sb, w in ((wg_sb, wg), (wu_sb, wu)):
                    for s0 in range(i * WBLK, (i + 1) * WBLK, hstep):
                        nc.sync.dma_start(
                            w_sb[:, s0 : s0 + hstep], w.ap()[:, s0 : s0 + hstep]
                        )
            nc.scalar.dma_start(cw_sb[:], cw.ap()[:])
            for a, b in ((0, 6), (6, IT)):
                nc.sync.dma_start(
                    wd_sb[:, a * H : b * H], wd.ap()[:, a * H : b * H]
                )

            def emit_matmul1(xt_sb):
                """silu(x@Wg) * (x@Wu) for one 512-token window -> gated^T.

                i-outer, k-inner: 16 consecutive matmuls accumulate into the
                SAME PSUM bank — cycling banks per-MM costs +43ns/MM (20%)
                in the PE pipeline, measured.
                """
                gated = []
                for i in range(IT):
                    pg = pgp.tile([128, 512], fp32, name="pg", tag="pg")
                    pu = pup.tile([128, 512], fp32, name="pu", tag="pu")
                    for k in range(KT):
                        nc.tensor.matmul(
                            pg[:],
                            wg_sb[:, i * WBLK + k * 128 : i * WBLK + (k + 1) * 128],
                            xt_sb[:, k * 512 : (k + 1) * 512],
                            start=(k == 0),
                            stop=(k == KT - 1),
                        )
                    for k in range(KT):
                        nc.tensor.matmul(
                            pu[:],
                            wu_sb[:, i * WBLK + k * 128 : i * WBLK + (k + 1) * 128],
                            xt_sb[:, k * 512 : (k + 1) * 512],
                            start=(k == 0),
                            stop=(k == KT - 1),
                        )
                    act = spool.tile([128, 512], fp32, name="act", tag="act")
                    nc.scalar.activation(act[:], pg[:], silu_fn)
                    g = gpool.tile([128, 512], fp16, name=f"g{i}", tag=f"g{i}")
                    nc.vector.tensor_tensor(g[:], act[:], pu[:], mult)
                    gated.append(g)
                return gated

            def emit_matmul2(wi, gated, last=False):
                # Down-proj: out[tokens, H] accumulated over I, then scaled by
                # the per-token combine weight into an fp16 staging slab,
                # written back with one DMA per m-tile (sync ring). The last
                # window issues per-h DMAs instead so the final transfer
                # trailing the last matmul is small.
                for m in range(4):
                    mg = wi * 4 + m
                    ob = opool.tile([128, HW * 512], fp16, name="ob", tag="ob")
                    for h in range(HW):
                        po = pop.tile([128, 512], fp32, name="po", tag="po")
                        for i in range(IT):
                            nc.tensor.matmul(
                                po[:],
                                gated[i][:, m * 128 : (m + 1) * 128],
                                wd_sb[:, i * H + h * 512 : i * H + (h + 1) * 512],
                                start=(i == 0),
                                stop=(i == IT - 1),
                            )
                        if last and mg == MT - 1 and h == HW - 1:
                            # Final output tile: split scale+DMA into 256-col
                            # halves so the first half's transfer (and its
                            # ~2us HBM write receipt) overlaps the second
                            # half's scale — shortens the critical chain
                            # from the kernel's last matmul to exec end.
                            for c, eng in ((0, nc.sync), (1, nc.scalar)):
                                lo = h * 512 + c * 256
                                nc.vector.tensor_scalar_mul(
                                    ob[:, lo : lo + 256],
                                    po[:, c * 256 : (c + 1) * 256],
                                    cw_sb[:, mg : mg + 1],
                                )
                                eng.dma_start(
                                    out.ap()[mg * 128 : (mg + 1) * 128, lo : lo + 256],
                                    ob[:, lo : lo + 256],
                                )
                            continue
                        nc.vector.tensor_scalar_mul(
                            ob[:, h * 512 : (h + 1) * 512], po[:], cw_sb[:, mg : mg + 1]
                        )
                        if last:
                            eng = nc.scalar if h % 2 else nc.sync
                            eng.dma_start(
                                out.ap()[
                                    mg * 128 : (mg + 1) * 128, h * 512 : (h + 1) * 512
                                ],
                                ob[:, h * 512 : (h + 1) * 512],
                            )
                    if not last:
                        nc.sync.dma_start(
                            out.ap()[mg * 128 : (mg + 1) * 128, :], ob[:]
                        )

            # Window pipeline: matmul2 of window t is emitted after matmul1 of
            # window t+1 (gpool bufs=2 keeps both windows' gated tiles live),
            # so the start-up down-matmuls don't stall on the wd load.
            pending = None
            for wi in range(NWIN):
                if wi == 0:
                    xt_sb = xt0_sb
                else:
                    xt_sb, xchunks = dma_xt_window(wi * 512, nsplit=1)
                    for c in xchunks:
                        # w1 rides the sync ring right behind wd (its xpool
                        # buf is free at t0); w2/w3 triggers block on buf
                        # release so they go on the scalar ring.
                        (nc.sync if wi == 1 else nc.scalar).dma_start(*c)
                gated = emit_matmul1(xt_sb)
                if pending is not None:
                    emit_matmul2(*pending)
                pending = (wi, gated)
            emit_matmul2(*pending, last=True)

    nc.compile()
    return nc


def kernel(
    hidden_states: np.ndarray,
    gate_w: np.ndarray,
    w_gate: np.ndarray,
    w_up: np.ndarray,
    w_down: np.ndarray,
) -> np.ndarray:
    from concourse.bass_utils import run_bass_kernel_spmd

    x = np.asarray(hidden_states, dtype=np.float32).reshape(-1, H)
    gate_w = np.asarray(gate_w, dtype=np.float32)
    w_gate = np.asarray(w_gate, dtype=np.float32)
    w_up = np.asarray(w_up, dtype=np.float32)
    w_down = np.asarray(w_down, dtype=np.float32)
    T = x.shape[0]

    # Router (the sharding decision): softmax over experts, top-2, renormalize.
    logits = x @ gate_w.T
    logits -= logits.max(axis=-1, keepdims=True)
    ex = np.exp(logits)
    probs = ex / ex.sum(axis=-1, keepdims=True)
    topk_i = np.argpartition(-probs, K - 1, axis=-1)[:, :K]  # [T, K]
    topk_w = np.take_along_axis(probs, topk_i, axis=-1)
    denom = topk_w.sum(axis=-1)  # [T]

    sels, cws, overflow = [], [], []
    for e in range(E):
        sel = np.nonzero((topk_i == e).any(axis=1))[0]
        cw_e = probs[sel, e] / denom[sel]
        if len(sel) > C:
            overflow.append((e, sel[C:], cw_e[C:]))
            sel, cw_e = sel[:C], cw_e[:C]
        sels.append(sel)
        cws.append(cw_e)

    if "nc" not in _NC_CACHE:
        _NC_CACHE["nc"] = _build_nc()
    nc = _NC_CACHE["nc"]

    # Dispatch: gather each expert's tokens (transposed, fp16) + weights,
    # swizzled into the SBUF-image layouts the kernel's DMAs expect.
    xt_full = np.ascontiguousarray(x.T.astype(np.float16))  # [H, T]

    def swz_w(w):  # [H, I] -> [128, IT*KT*128] i-block-major image
        return np.ascontiguousarray(
            w.astype(np.float16)
            .reshape(KT, 128, IT, 128)
            .transpose(1, 2, 0, 3)
            .reshape(128, IT * KT * 128)
        )

    def swz_wd(w):  # [I, H] -> [128, IT*H] i-block-major image
        return np.ascontiguousarray(
            w.astype(np.float16).reshape(IT, 128, H).transpose(1, 0, 2).reshape(128, IT * H)
        )

    def swz_xt(xpad):  # [H, C] -> [128, KT*C] window-major image
        blocks = [
            xpad[:, o : o + 512].reshape(KT, 128, 512).transpose(1, 0, 2).reshape(128, -1)
            for o in range(0, C, 512)
        ]
        return np.ascontiguousarray(np.concatenate(blocks, axis=1))

    in_maps = []
    for e in range(E):
        sel = sels[e]
        xpad = np.zeros((H, C), dtype=np.float16)
        xpad[:, : len(sel)] = xt_full[:, sel]
        cw_e = np.zeros((128, MT), dtype=np.float32)
        cw_flat = np.zeros(MT * 128, dtype=np.float32)
        cw_flat[: len(sel)] = cws[e]
        cw_e[:] = cw_flat.reshape(MT, 128).T
        in_maps.append(
            {
                "xt": swz_xt(xpad),
                "wg": swz_w(w_gate[e]),
                "wu": swz_w(w_up[e]),
                "wd": swz_wd(w_down[e]),
                "cw": cw_e,
            }
        )

    trace = bool(os.environ.get("BASS_MOE_TRACE"))
    res = run_bass_kernel_spmd(
        nc, in_maps, core_ids=list(range(N_CORES)), trace=trace
    )
    if trace and res.exec_time_ns is not None:
        print(f"HW exec time: {res.exec_time_ns} ns")

    # Combine: scatter-add each expert's (already weight-scaled) rows.
    out_full = np.zeros((T, H), dtype=np.float32)
    for e in range(E):
        sel = sels[e]
        out_full[sel] += res.results[e]["out"][: len(sel)].astype(np.float32)
    # Token-expert pairs beyond a hot expert's capacity: exact fp32 on host
    # (~0.6% of pairs for this router distribution).
    for e, sel, cw_e in overflow:
        xs = x[sel]
        g = xs @ w_gate[e]
        gated = (g / (1.0 + np.exp(-g))) * (xs @ w_up[e])
        out_full[sel] += cw_e[:, None] * (gated @ w_down[e])
    return out_full.reshape(B, S, H)
